# revision 16
# baseline (speedup 1.0000x reference)
"""Trainium2 Bass kernel: EquivariantLayerNorm (irreps 128x0e + 64x1o + 32x2e).

Math (per row x of 480 features; scalar channels = first 128):
    m    = mean(x[:128]);  x'[:128] = x[:128] - m;  x'[128:] = x[128:]
    ss   = sum(x'^2) = sum(x^2) - 128*m^2           (groups partition the row)
    r    = 1/sqrt(ss/224)
    y    = x' * r * w_full;  y[:128] += bias
The Invariant eps terms (eps=1e-6) contribute <1e-6 relative error and are
dropped (below fp32 rounding of the reference itself).

I/O in fp16: the host casts inputs to fp16 and upconverts the fp16 output;
this halves HBM traffic (the kernel is memory-bound) and lands well inside
the 2e-2 relative-error budget (~9e-4 measured on HW).

Sharding: pure data-parallel over rows across 8 NeuronCores; each core gets
12500 rows padded to 12544 = 98 blocks of 128 rows (pad rows 1.0).

Per-core layout: rows on partitions, features on the free dim. Tiles of g
row-blocks per DMA (G_SCHEDULE), mapped "(p g) d -> p g d" so each partition
holds g CONSECUTIVE HBM rows: one contiguous ~g*960B descriptor per
partition per tile instead of g small ones (8x fewer DMA descriptors; loads
were descriptor-bound at ~98 GB/s with the interleaved mapping).

GPSIMD is kept COMPLETELY IDLE: profiling showed GPSIMD activity (elementwise
TTs + SWDGE descriptor generation) contends with DVE on the shared SBUF port
and inflates concurrent DVE ops up to 8x. Loads issue on the SP HWDGE ring,
stores (and the one-time w/b broadcasts) on the ACT HWDGE ring.

Emission is software-pipelined (stats front-half of tile t+1 before the
scale phase of tile t) so neither DVE nor ACT stalls on the cross-engine
stats chain (ACT square-accums -> DVE vv -> ACT sqrt -> DVE recip):
  DVE : s1 rowsum (one 3D tensor_reduce per tile); per-block 2-scalar
        tensor_scalar (x+nm)*rr on the scalar chunk and *rr on the rest
        (2x fp16); whole-tile tensor_tensor *w and +bias (2x fp16);
        small stats ops; reciprocal; ss for SS_SPLIT blocks/tile.
  ACT : per-block Square-accumulate -> sum(x^2)/224 for the other blocks;
        batched Square for the DVE-split blocks; sqrt; store dma_start.
  SP  : input loads (HWDGE).
"""

import numpy as np

DIM = 480
NS = 128          # scalar (0e) channels
NF = 224          # irrep instances
BLK = 128         # rows per block (partition dim)
N_CORES = 8

N_TOTAL = 100000
ROWS_PER_CORE = N_TOTAL // N_CORES    # 12500
NBLOCKS = 98                          # ceil(12500/128)
ROWS_PAD = NBLOCKS * BLK              # 12544

# Variable tile schedule: small first tiles start compute early; small last
# tiles trim the drain tail (the final tile's serial chain).
G_SCHEDULE = (4, 4) + (8,) * 10 + (4, 4, 2)   # sums to 98
SS_SPLIT = 1     # blocks per tile whose sum-sq runs on DVE (ACT Square into
                 # xsq scratch, then DVE tensor_scalar-accumulate); the rest
                 # use ACT Square-accumulate. Balances ACT/DVE load.


def _expand_w(affine_weight):
    return np.concatenate([
        affine_weight[0:128],
        np.repeat(affine_weight[128:192], 3),
        np.repeat(affine_weight[192:224], 5),
    ]).astype(np.float32)


def _split_excess_waits(nc, dummy_sem):
    """walrus' TRN2 codegen allows at most ONE sync-wait command per engine
    instruction (S3D3_*_STRUCT).  Tile's wait assignment can emit 2+ — move
    the excess onto standalone InstEventSemaphore no-ops (same engine, placed
    immediately before), which is the same mechanism Tile's own barriers use.
    Each carries a dead increment of ``dummy_sem`` (CoreSim requires updates).
    """
    from concourse import mybir

    n = 0
    for fn in nc.m.functions:
        for blk in fn.blocks:
            out = []
            changed = False
            for inst in blk.instructions:
                si = inst.sync_info
                if si is not None and si.on_wait and len(si.on_wait) > 1:
                    waits = list(si.on_wait)
                    for w in waits[:-1]:
                        n += 1
                        ev = mybir.InstEventSemaphore(
                            name=f"I-evsplit-{n}", ins=[], outs=[])
                        ev.engine = inst.engine
                        ev.sync_info = mybir.SyncInfo(
                            on_wait=[w],
                            on_update=[mybir.SyncUpdate(
                                sync_type="semaphore", id=dummy_sem.num,
                                ant_name=dummy_sem.name,
                                update_mode="sem-inc", update_value=1,
                                update_reg=None)])
                        out.append(ev)
                    inst.sync_info = mybir.SyncInfo(
                        on_wait=[waits[-1]], on_update=list(si.on_update or []))
                    changed = True
                out.append(inst)
            if changed:
                blk.instructions = out
    return n


def build_nc(rows_pad, g, data_bufs=8, ss_split=SS_SPLIT):
    import concourse.bacc as bacc
    import concourse.tile as tile
    from concourse import mybir
    # Loads issue from the SP HWDGE ring; stores from the ACT HWDGE ring so
    # the two directions run on independent FIFO rings and GPSIMD (whose
    # SWDGE descriptor generation would contend with DVE on the shared SBUF
    # port) stays idle. Pin one completion-semaphore lane per ring.
    from concourse import tile_sem_assignment as _tsa
    if not getattr(_tsa.TileClockTick, "_lane_by_engine", False):
        _orig_assign = _tsa.TileClockTick._assign_tick

        def _assign_tick_lane_by_engine(self, inst):
            if isinstance(inst, _tsa.DMAInst) and not isinstance(
                    inst, _tsa.bass_isa.UserSyncedRemoteDMADescs):
                if inst.engine == mybir.EngineType.SP:
                    self.next_hw_dma_idx = 0
                elif inst.engine == mybir.EngineType.Activation:
                    self.next_hw_dma_idx = 1
            return _orig_assign(self, inst)

        _tsa.TileClockTick._assign_tick = _assign_tick_lane_by_engine
        _tsa.TileClockTick._lane_by_engine = True

    f16 = mybir.dt.float16
    f32 = mybir.dt.float32
    Alu = mybir.AluOpType
    Act = mybir.ActivationFunctionType

    nblocks = rows_pad // BLK
    assert rows_pad % BLK == 0
    if isinstance(g, int):
        assert nblocks % g == 0
        gs = [g] * (nblocks // g)
    else:
        gs = list(g)
        assert sum(gs) == nblocks
    ntiles = len(gs)
    r0s = list(np.cumsum([0] + gs[:-1]) * BLK)

    nc = bacc.Bacc("TRN2", target_bir_lowering=False, debug=False)
    evsem = nc.alloc_semaphore("evsplit_dummy")
    x = nc.dram_tensor("x", [rows_pad, DIM], f16, kind="ExternalInput").ap()
    w = nc.dram_tensor("w", [1, DIM], f16, kind="ExternalInput").ap()
    b = nc.dram_tensor("b", [1, NS], f16, kind="ExternalInput").ap()
    y = nc.dram_tensor("y", [rows_pad, DIM], f16, kind="ExternalOutput").ap()

    with tile.TileContext(nc) as tc:
        with (
            tc.tile_pool(name="const", bufs=1) as cpool,
            tc.tile_pool(name="data", bufs=data_bufs) as dpool,
            tc.tile_pool(name="stats", bufs=ntiles) as spool,
            tc.tile_pool(name="scratch", bufs=1) as zpool,
            tc.tile_pool(name="sq", bufs=3) as qpool,
        ):
            # One-time broadcast loads on the (initially idle) ACT ring so
            # they don't clog the SP ring ahead of the first data tiles.
            w_t = cpool.tile([BLK, DIM], f16, name="w_t")
            nc.scalar.dma_start(out=w_t[:], in_=w.broadcast_to([BLK, DIM]))
            b_t = cpool.tile([BLK, NS], f16, name="b_t")
            nc.scalar.dma_start(out=b_t[:], in_=b.broadcast_to([BLK, NS]))

            df = zpool.tile([BLK, DIM], f16, name="df")   # ACT dead (squares)
            da = zpool.tile([BLK, DIM], f16, name="da")   # DVE dead (TS accum)

            T = [dict() for _ in range(ntiles)]

            def emit_load(t):
                g = gs[t]
                src = x[r0s[t]:r0s[t] + g * BLK, :].rearrange(
                    "(p g) d -> p g d", p=BLK)
                xt = dpool.tile([BLK, g, DIM], f16, tag="xt", name=f"xt{t}")
                # Ramp: odd early tiles load via the (otherwise idle) ACT
                # HWDGE ring so both rings stream in parallel and the first
                # tiles arrive ~2x sooner; steady state stays on SP.
                if t in (1, 3):
                    nc.scalar.dma_start(out=xt[:], in_=src)
                else:
                    nc.sync.dma_start(out=xt[:], in_=src)
                T[t]["xt"] = xt

            def emit_reduce_nm(t):
                g, xt = gs[t], T[t]["xt"]
                s1r = spool.tile([BLK, g], f32, tag="s1r", name=f"s1r{t}")
                nc.vector.tensor_reduce(
                    out=s1r[:], in_=xt[:, :, 0:NS],
                    axis=mybir.AxisListType.X, op=Alu.add)
                nm = spool.tile([BLK, g], f32, tag="nm", name=f"nm{t}")
                nc.vector.tensor_scalar(
                    out=nm[:], in0=s1r[:], scalar1=-1.0 / NS, scalar2=None,
                    op0=Alu.mult)
                T[t]["nm"] = nm

            def emit_accums(t):
                g, xt = gs[t], T[t]["xt"]
                ss = spool.tile([BLK, g], f32, tag="ss", name=f"ss{t}")
                k = min(ss_split, g)
                for j in range(g - k):
                    nc.scalar.activation(
                        out=df[:], in_=xt[:, j, :], func=Act.Square,
                        scale=1.0 / float(NF) ** 0.5,
                        accum_out=ss[:, j:j + 1])
                if k > 0:
                    xsq = qpool.tile([BLK, k, DIM], f16, tag="xsq",
                                     name=f"xsq{t}")
                    nc.scalar.activation(
                        out=xsq[:], in_=xt[:, g - k:, :], func=Act.Square,
                        scale=1.0 / float(NF) ** 0.5)
                    T[t]["xsq"] = xsq
                T[t]["ss"] = ss
                T[t]["k"] = k

            def emit_mid(t):
                g, nm, ss, k = gs[t], T[t]["nm"], T[t]["ss"], T[t]["k"]
                for j in range(k):
                    # op1 is the reduce op when accum_out is given
                    nc.vector.tensor_scalar(
                        out=da[:], in0=T[t]["xsq"][:, j, :], scalar1=1.0,
                        scalar2=None, op0=Alu.mult, op1=Alu.add,
                        accum_out=ss[:, g - k + j:g - k + j + 1])
                # vv = ss/224 - (128/224)*nm^2 ; rr = 1/sqrt(vv)
                u = spool.tile([BLK, g], f32, tag="u", name=f"u{t}")
                nc.vector.scalar_tensor_tensor(
                    out=u[:], in0=nm[:], scalar=-float(NS) / float(NF),
                    in1=nm[:], op0=Alu.mult, op1=Alu.mult)
                vv = spool.tile([BLK, g], f32, tag="vv", name=f"vv{t}")
                nc.vector.tensor_tensor(
                    out=vv[:], in0=ss[:], in1=u[:], op=Alu.add)
                sq = spool.tile([BLK, g], f32, tag="sq", name=f"sq{t}")
                nc.scalar.activation(out=sq[:], in_=vv[:], func=Act.Sqrt)
                rr = spool.tile([BLK, g], f32, tag="rr", name=f"rr{t}")
                nc.vector.reciprocal(out=rr[:], in_=sq[:])
                T[t]["rr"] = rr

            def emit_scale(t):
                g, xt, nm, rr = gs[t], T[t]["xt"], T[t]["nm"], T[t]["rr"]
                # Chunk A: (x + nm) * rr fused in one 2-scalar TS;
                # chunks B/C: x * rr. Both 2x fp16 on DVE.
                for j in range(g):
                    nc.vector.tensor_scalar(
                        out=xt[:, j, 0:NS], in0=xt[:, j, 0:NS],
                        scalar1=nm[:, j:j + 1], scalar2=rr[:, j:j + 1],
                        op0=Alu.add, op1=Alu.mult)
                    nc.vector.tensor_scalar(
                        out=xt[:, j, NS:DIM], in0=xt[:, j, NS:DIM],
                        scalar1=rr[:, j:j + 1], scalar2=None, op0=Alu.mult)
                # Whole-tile *w, then whole-tile +bias on the scalar chunk.
                w_bc = w_t[:].unsqueeze(1).broadcast_to([BLK, g, DIM])
                nc.vector.tensor_tensor(
                    out=xt[:], in0=xt[:], in1=w_bc, op=Alu.mult)
                b_bc = b_t[:].unsqueeze(1).broadcast_to([BLK, g, NS])
                nc.vector.tensor_tensor(
                    out=xt[:, :, 0:NS], in0=xt[:, :, 0:NS], in1=b_bc,
                    op=Alu.add)

            def emit_store(t):
                # SWDGE store: GPSIMD has no other work, so the store's wait
                # for the tile's last DVE op costs nothing — a waiting store
                # on the FIFO ACT ring would stall the accums/sqrt behind it.
                g, xt = gs[t], T[t]["xt"]
                dst = y[r0s[t]:r0s[t] + g * BLK, :].rearrange(
                    "(p g) d -> p g d", p=BLK)
                nc.gpsimd.dma_start(out=dst, in_=xt[:])

            # Software-pipelined emission: per-engine instruction order
            # follows emission order, so putting tile t+1's load/reduce
            # ahead of tile t's mid/scale keeps both engines fed.
            emit_load(0)
            emit_reduce_nm(0)
            for t in range(ntiles):
                if t + 1 < ntiles:
                    emit_load(t + 1)
                    emit_reduce_nm(t + 1)
                emit_accums(t)
                emit_mid(t)
                emit_scale(t)
                emit_store(t)

    nc.compile()
    _split_excess_waits(nc, evsem)
    return nc


_NC_CACHE = {}


def _get_nc(rows_pad, g):
    key = (rows_pad, tuple(g) if not isinstance(g, int) else g)
    if key not in _NC_CACHE:
        _NC_CACHE[key] = build_nc(rows_pad, g)
    return _NC_CACHE[key]


def kernel(node_input, affine_weight, affine_bias):
    from concourse.bass_utils import run_bass_kernel_spmd

    x16 = np.asarray(node_input).astype(np.float16)
    assert x16.shape == (N_TOTAL, DIM)
    w16 = _expand_w(np.asarray(affine_weight, dtype=np.float32)).astype(
        np.float16).reshape(1, DIM)
    b16 = np.asarray(affine_bias).astype(np.float16).reshape(1, NS)

    in_maps = []
    for c in range(N_CORES):
        shard = np.ones((ROWS_PAD, DIM), dtype=np.float16)
        shard[:ROWS_PER_CORE] = x16[c * ROWS_PER_CORE:(c + 1) * ROWS_PER_CORE]
        in_maps.append({"x": shard, "w": w16, "b": b16})

    nc = _get_nc(ROWS_PAD, G_SCHEDULE)
    res = run_bass_kernel_spmd(nc, in_maps, core_ids=list(range(N_CORES)))
    out = np.concatenate(
        [np.asarray(res.results[c]["y"])[:ROWS_PER_CORE] for c in range(N_CORES)],
        axis=0)
    return out.astype(np.float32)


# revision 19
# speedup vs baseline: 1.0458x; 1.0458x over previous
"""Trainium2 Bass kernel: EquivariantLayerNorm (irreps 128x0e + 64x1o + 32x2e).

Math (per row x of 480 features; scalar channels = first 128):
    m    = mean(x[:128]);  x'[:128] = x[:128] - m;  x'[128:] = x[128:]
    ss   = sum(x'^2) = sum(x^2) - 128*m^2           (groups partition the row)
    r    = 1/sqrt(ss/224)
    y    = x' * r * w_full;  y[:128] += bias
The Invariant eps terms (eps=1e-6) contribute <1e-6 relative error and are
dropped (below fp32 rounding of the reference itself).

I/O in fp16: the host casts inputs to fp16 and upconverts the fp16 output;
this halves HBM traffic (the kernel is memory-bound) and lands well inside
the 2e-2 relative-error budget (~9e-4 measured on HW).

Sharding: pure data-parallel over rows across 8 NeuronCores; each core gets
12500 rows padded to 12544 = 98 blocks of 128 rows (pad rows 1.0).

Per-core layout: rows on partitions, features on the free dim. Tiles of g
row-blocks per DMA (G_SCHEDULE), mapped "(p g) d -> p g d" so each partition
holds g CONSECUTIVE HBM rows: one contiguous ~g*960B descriptor per
partition per tile instead of g small ones (8x fewer DMA descriptors; loads
were descriptor-bound at ~98 GB/s with the interleaved mapping).

GPSIMD is kept COMPLETELY IDLE: profiling showed GPSIMD activity (elementwise
TTs + SWDGE descriptor generation) contends with DVE on the shared SBUF port
and inflates concurrent DVE ops up to 8x. Loads issue on the SP HWDGE ring,
stores (and the one-time w/b broadcasts) on the ACT HWDGE ring.

Emission is software-pipelined (stats front-half of tile t+1 before the
scale phase of tile t) so neither DVE nor ACT stalls on the cross-engine
stats chain (ACT square-accums -> DVE vv -> ACT sqrt -> DVE recip):
  DVE : s1 rowsum (one 3D tensor_reduce per tile); per-block 2-scalar
        tensor_scalar (x+nm)*rr on the scalar chunk and *rr on the rest
        (2x fp16); whole-tile tensor_tensor *w and +bias (2x fp16);
        small stats ops; reciprocal; ss for SS_SPLIT blocks/tile.
  ACT : per-block Square-accumulate -> sum(x^2)/224 for the other blocks;
        batched Square for the DVE-split blocks; sqrt; store dma_start.
  SP  : input loads (HWDGE).
"""

import numpy as np

DIM = 480
NS = 128          # scalar (0e) channels
NF = 224          # irrep instances
BLK = 128         # rows per block (partition dim)
N_CORES = 8

N_TOTAL = 100000
ROWS_PER_CORE = N_TOTAL // N_CORES    # 12500
NBLOCKS = 98                          # ceil(12500/128)
ROWS_PAD = NBLOCKS * BLK              # 12544

# Variable tile schedule: small first tiles start compute early; small last
# tiles trim the drain tail (the final tile's serial chain).
G_SCHEDULE = (4, 4) + (8,) * 10 + (4, 4, 2)   # sums to 98
SS_SPLIT = 1     # blocks per tile whose sum-sq runs on DVE (ACT Square into
                 # xsq scratch, then DVE tensor_scalar-accumulate); the rest
                 # use ACT Square-accumulate. Balances ACT/DVE load.


def _expand_w(affine_weight):
    return np.concatenate([
        affine_weight[0:128],
        np.repeat(affine_weight[128:192], 3),
        np.repeat(affine_weight[192:224], 5),
    ]).astype(np.float32)


def _split_excess_waits(nc, dummy_sem):
    """walrus' TRN2 codegen allows at most ONE sync-wait command per engine
    instruction (S3D3_*_STRUCT).  Tile's wait assignment can emit 2+ — move
    the excess onto standalone InstEventSemaphore no-ops (same engine, placed
    immediately before), which is the same mechanism Tile's own barriers use.
    Each carries a dead increment of ``dummy_sem`` (CoreSim requires updates).
    """
    from concourse import mybir

    n = 0
    for fn in nc.m.functions:
        for blk in fn.blocks:
            out = []
            changed = False
            for inst in blk.instructions:
                si = inst.sync_info
                if si is not None and si.on_wait and len(si.on_wait) > 1:
                    waits = list(si.on_wait)
                    for w in waits[:-1]:
                        n += 1
                        ev = mybir.InstEventSemaphore(
                            name=f"I-evsplit-{n}", ins=[], outs=[])
                        ev.engine = inst.engine
                        ev.sync_info = mybir.SyncInfo(
                            on_wait=[w],
                            on_update=[mybir.SyncUpdate(
                                sync_type="semaphore", id=dummy_sem.num,
                                ant_name=dummy_sem.name,
                                update_mode="sem-inc", update_value=1,
                                update_reg=None)])
                        out.append(ev)
                    inst.sync_info = mybir.SyncInfo(
                        on_wait=[waits[-1]], on_update=list(si.on_update or []))
                    changed = True
                out.append(inst)
            if changed:
                blk.instructions = out
    return n


def build_nc(rows_pad, g, data_bufs=8, ss_split=SS_SPLIT):
    import concourse.bacc as bacc
    import concourse.tile as tile
    from concourse import mybir
    # Loads issue from the SP HWDGE ring; stores from the ACT HWDGE ring so
    # the two directions run on independent FIFO rings and GPSIMD (whose
    # SWDGE descriptor generation would contend with DVE on the shared SBUF
    # port) stays idle. Pin one completion-semaphore lane per ring.
    from concourse import tile_sem_assignment as _tsa
    if not getattr(_tsa.TileClockTick, "_lane_by_engine", False):
        _orig_assign = _tsa.TileClockTick._assign_tick

        def _assign_tick_lane_by_engine(self, inst):
            if isinstance(inst, _tsa.DMAInst) and not isinstance(
                    inst, _tsa.bass_isa.UserSyncedRemoteDMADescs):
                if inst.engine == mybir.EngineType.SP:
                    self.next_hw_dma_idx = 0
                elif inst.engine == mybir.EngineType.Activation:
                    self.next_hw_dma_idx = 1
            return _orig_assign(self, inst)

        _tsa.TileClockTick._assign_tick = _assign_tick_lane_by_engine
        _tsa.TileClockTick._lane_by_engine = True

    f16 = mybir.dt.float16
    f32 = mybir.dt.float32
    Alu = mybir.AluOpType
    Act = mybir.ActivationFunctionType

    nblocks = rows_pad // BLK
    assert rows_pad % BLK == 0
    if isinstance(g, int):
        assert nblocks % g == 0
        gs = [g] * (nblocks // g)
    else:
        gs = list(g)
        assert sum(gs) == nblocks
    ntiles = len(gs)
    r0s = list(np.cumsum([0] + gs[:-1]) * BLK)

    nc = bacc.Bacc("TRN2", target_bir_lowering=False, debug=False)
    evsem = nc.alloc_semaphore("evsplit_dummy")
    x = nc.dram_tensor("x", [rows_pad, DIM], f16, kind="ExternalInput").ap()
    # w/b arrive pre-replicated across the 128 partitions (host-side tile):
    # a plain contiguous load is ~20x faster than a broadcast DMA (128 tiny
    # descriptors) and keeps the ACT ring free for the ramp loads behind it.
    w = nc.dram_tensor("w", [BLK, DIM], f16, kind="ExternalInput").ap()
    b = nc.dram_tensor("b", [BLK, NS], f16, kind="ExternalInput").ap()
    y = nc.dram_tensor("y", [rows_pad, DIM], f16, kind="ExternalOutput").ap()

    with tile.TileContext(nc) as tc:
        with (
            tc.tile_pool(name="const", bufs=1) as cpool,
            tc.tile_pool(name="data", bufs=data_bufs) as dpool,
            tc.tile_pool(name="stats", bufs=ntiles) as spool,
            tc.tile_pool(name="scratch", bufs=1) as zpool,
            tc.tile_pool(name="sq", bufs=3) as qpool,
        ):
            # One-time const loads on the (initially idle) ACT ring so they
            # don't clog the SP ring ahead of the first data tiles.
            w_t = cpool.tile([BLK, DIM], f16, name="w_t")
            nc.scalar.dma_start(out=w_t[:], in_=w)
            b_t = cpool.tile([BLK, NS], f16, name="b_t")
            nc.scalar.dma_start(out=b_t[:], in_=b)

            df = zpool.tile([BLK, DIM], f16, name="df")   # ACT dead (squares)
            da = zpool.tile([BLK, DIM], f16, name="da")   # DVE dead (TS accum)

            T = [dict() for _ in range(ntiles)]

            def emit_load(t):
                g = gs[t]
                src = x[r0s[t]:r0s[t] + g * BLK, :].rearrange(
                    "(p g) d -> p g d", p=BLK)
                xt = dpool.tile([BLK, g, DIM], f16, tag="xt", name=f"xt{t}")
                # Ramp: odd early tiles load via the (otherwise idle) ACT
                # HWDGE ring so both rings stream in parallel and the first
                # tiles arrive ~2x sooner; steady state stays on SP.
                if t in (1, 3):
                    nc.scalar.dma_start(out=xt[:], in_=src)
                else:
                    nc.sync.dma_start(out=xt[:], in_=src)
                T[t]["xt"] = xt

            def emit_reduce_nm(t):
                g, xt = gs[t], T[t]["xt"]
                s1r = spool.tile([BLK, g], f32, tag="s1r", name=f"s1r{t}")
                nc.vector.tensor_reduce(
                    out=s1r[:], in_=xt[:, :, 0:NS],
                    axis=mybir.AxisListType.X, op=Alu.add)
                nm = spool.tile([BLK, g], f32, tag="nm", name=f"nm{t}")
                nc.vector.tensor_scalar(
                    out=nm[:], in0=s1r[:], scalar1=-1.0 / NS, scalar2=None,
                    op0=Alu.mult)
                T[t]["nm"] = nm

            def emit_accums(t):
                g, xt = gs[t], T[t]["xt"]
                ss = spool.tile([BLK, g], f32, tag="ss", name=f"ss{t}")
                k = min(ss_split, g)
                for j in range(g - k):
                    nc.scalar.activation(
                        out=df[:], in_=xt[:, j, :], func=Act.Square,
                        scale=1.0 / float(NF) ** 0.5,
                        accum_out=ss[:, j:j + 1])
                if k > 0:
                    xsq = qpool.tile([BLK, k, DIM], f16, tag="xsq",
                                     name=f"xsq{t}")
                    nc.scalar.activation(
                        out=xsq[:], in_=xt[:, g - k:, :], func=Act.Square,
                        scale=1.0 / float(NF) ** 0.5)
                    T[t]["xsq"] = xsq
                T[t]["ss"] = ss
                T[t]["k"] = k

            def emit_mid(t):
                g, nm, ss, k = gs[t], T[t]["nm"], T[t]["ss"], T[t]["k"]
                for j in range(k):
                    # op1 is the reduce op when accum_out is given
                    nc.vector.tensor_scalar(
                        out=da[:], in0=T[t]["xsq"][:, j, :], scalar1=1.0,
                        scalar2=None, op0=Alu.mult, op1=Alu.add,
                        accum_out=ss[:, g - k + j:g - k + j + 1])
                # vv = ss/224 - (128/224)*nm^2 ; rr = 1/sqrt(vv)
                u = spool.tile([BLK, g], f32, tag="u", name=f"u{t}")
                nc.vector.scalar_tensor_tensor(
                    out=u[:], in0=nm[:], scalar=-float(NS) / float(NF),
                    in1=nm[:], op0=Alu.mult, op1=Alu.mult)
                vv = spool.tile([BLK, g], f32, tag="vv", name=f"vv{t}")
                nc.vector.tensor_tensor(
                    out=vv[:], in0=ss[:], in1=u[:], op=Alu.add)
                sq = spool.tile([BLK, g], f32, tag="sq", name=f"sq{t}")
                nc.scalar.activation(out=sq[:], in_=vv[:], func=Act.Sqrt)
                rr = spool.tile([BLK, g], f32, tag="rr", name=f"rr{t}")
                nc.vector.reciprocal(out=rr[:], in_=sq[:])
                T[t]["rr"] = rr

            def emit_scale(t):
                g, xt, nm, rr = gs[t], T[t]["xt"], T[t]["nm"], T[t]["rr"]
                # Chunk A: (x + nm) * rr fused in one 2-scalar TS;
                # chunks B/C: x * rr. Both 2x fp16 on DVE.
                for j in range(g):
                    nc.vector.tensor_scalar(
                        out=xt[:, j, 0:NS], in0=xt[:, j, 0:NS],
                        scalar1=nm[:, j:j + 1], scalar2=rr[:, j:j + 1],
                        op0=Alu.add, op1=Alu.mult)
                    nc.vector.tensor_scalar(
                        out=xt[:, j, NS:DIM], in0=xt[:, j, NS:DIM],
                        scalar1=rr[:, j:j + 1], scalar2=None, op0=Alu.mult)
                # Whole-tile *w, then whole-tile +bias on the scalar chunk.
                w_bc = w_t[:].unsqueeze(1).broadcast_to([BLK, g, DIM])
                nc.vector.tensor_tensor(
                    out=xt[:], in0=xt[:], in1=w_bc, op=Alu.mult)
                b_bc = b_t[:].unsqueeze(1).broadcast_to([BLK, g, NS])
                nc.vector.tensor_tensor(
                    out=xt[:, :, 0:NS], in0=xt[:, :, 0:NS], in1=b_bc,
                    op=Alu.add)

            def emit_store(t):
                # SWDGE store: GPSIMD has no other work, so the store's wait
                # for the tile's last DVE op costs nothing — a waiting store
                # on the FIFO ACT ring would stall the accums/sqrt behind it.
                g, xt = gs[t], T[t]["xt"]
                dst = y[r0s[t]:r0s[t] + g * BLK, :].rearrange(
                    "(p g) d -> p g d", p=BLK)
                nc.gpsimd.dma_start(out=dst, in_=xt[:])

            # Software-pipelined emission: per-engine instruction order
            # follows emission order, so putting tile t+1's load/reduce
            # ahead of tile t's mid/scale keeps both engines fed.
            emit_load(0)
            emit_reduce_nm(0)
            for t in range(ntiles):
                if t + 1 < ntiles:
                    emit_load(t + 1)
                    emit_reduce_nm(t + 1)
                emit_accums(t)
                emit_mid(t)
                emit_scale(t)
                emit_store(t)

    nc.compile()
    _split_excess_waits(nc, evsem)
    return nc


_NC_CACHE = {}


def _get_nc(rows_pad, g):
    key = (rows_pad, tuple(g) if not isinstance(g, int) else g)
    if key not in _NC_CACHE:
        _NC_CACHE[key] = build_nc(rows_pad, g)
    return _NC_CACHE[key]


def kernel(node_input, affine_weight, affine_bias):
    from concourse.bass_utils import run_bass_kernel_spmd

    x16 = np.asarray(node_input).astype(np.float16)
    assert x16.shape == (N_TOTAL, DIM)
    w16 = np.tile(_expand_w(np.asarray(affine_weight, dtype=np.float32)).astype(
        np.float16).reshape(1, DIM), (BLK, 1))
    b16 = np.tile(np.asarray(affine_bias).astype(np.float16).reshape(1, NS),
                  (BLK, 1))

    in_maps = []
    for c in range(N_CORES):
        shard = np.ones((ROWS_PAD, DIM), dtype=np.float16)
        shard[:ROWS_PER_CORE] = x16[c * ROWS_PER_CORE:(c + 1) * ROWS_PER_CORE]
        in_maps.append({"x": shard, "w": w16, "b": b16})

    nc = _get_nc(ROWS_PAD, G_SCHEDULE)
    res = run_bass_kernel_spmd(nc, in_maps, core_ids=list(range(N_CORES)))
    out = np.concatenate(
        [np.asarray(res.results[c]["y"])[:ROWS_PER_CORE] for c in range(N_CORES)],
        axis=0)
    return out.astype(np.float32)


# revision 21
# speedup vs baseline: 1.0628x; 1.0162x over previous
"""Trainium2 Bass kernel: EquivariantLayerNorm (irreps 128x0e + 64x1o + 32x2e).

Math (per row x of 480 features; scalar channels = first 128):
    m    = mean(x[:128]);  x'[:128] = x[:128] - m;  x'[128:] = x[128:]
    ss   = sum(x'^2) = sum(x^2) - 128*m^2           (groups partition the row)
    r    = 1/sqrt(ss/224)
    y    = x' * r * w_full;  y[:128] += bias
The Invariant eps terms (eps=1e-6) contribute <1e-6 relative error and are
dropped (below fp32 rounding of the reference itself).

I/O in fp16: the host casts inputs to fp16 and upconverts the fp16 output;
this halves HBM traffic (the kernel is memory-bound) and lands well inside
the 2e-2 relative-error budget (~9e-4 measured on HW).

Sharding: pure data-parallel over rows across 8 NeuronCores; each core gets
12500 rows padded to 12544 = 98 blocks of 128 rows (pad rows 1.0).

Per-core layout: rows on partitions, features on the free dim. Tiles of g
row-blocks per DMA (G_SCHEDULE), mapped "(p g) d -> p g d" so each partition
holds g CONSECUTIVE HBM rows: one contiguous ~g*960B descriptor per
partition per tile instead of g small ones (8x fewer DMA descriptors; loads
were descriptor-bound at ~98 GB/s with the interleaved mapping).

GPSIMD is kept COMPLETELY IDLE: profiling showed GPSIMD activity (elementwise
TTs + SWDGE descriptor generation) contends with DVE on the shared SBUF port
and inflates concurrent DVE ops up to 8x. Loads issue on the SP HWDGE ring,
stores (and the one-time w/b broadcasts) on the ACT HWDGE ring.

Emission is software-pipelined (stats front-half of tile t+1 before the
scale phase of tile t) so neither DVE nor ACT stalls on the cross-engine
stats chain (ACT square-accums -> DVE vv -> ACT sqrt -> DVE recip):
  DVE : s1 rowsum (one 3D tensor_reduce per tile); per-block 2-scalar
        tensor_scalar (x+nm)*rr on the scalar chunk and *rr on the rest
        (2x fp16); whole-tile tensor_tensor *w and +bias (2x fp16);
        small stats ops; reciprocal; ss for SS_SPLIT blocks/tile.
  ACT : per-block Square-accumulate -> sum(x^2)/224 for the other blocks;
        batched Square for the DVE-split blocks; sqrt; store dma_start.
  SP  : input loads (HWDGE).
"""

import numpy as np

DIM = 480
NS = 128          # scalar (0e) channels
NF = 224          # irrep instances
BLK = 128         # rows per block (partition dim)
N_CORES = 8

N_TOTAL = 100000
ROWS_PER_CORE = N_TOTAL // N_CORES    # 12500
NBLOCKS = 98                          # ceil(12500/128)
ROWS_PAD = NBLOCKS * BLK              # 12544

# Variable tile schedule: small first tiles start compute early; small last
# tiles trim the drain tail (the final tile's serial chain).
G_SCHEDULE = (6,) + (8,) * 11 + (4,)   # sums to 98
SS_SPLIT = 1     # blocks per tile whose sum-sq runs on DVE (ACT Square into
                 # xsq scratch, then DVE tensor_scalar-accumulate); the rest
                 # use ACT Square-accumulate. Balances ACT/DVE load.


def _expand_w(affine_weight):
    return np.concatenate([
        affine_weight[0:128],
        np.repeat(affine_weight[128:192], 3),
        np.repeat(affine_weight[192:224], 5),
    ]).astype(np.float32)


def _split_excess_waits(nc, dummy_sem):
    """walrus' TRN2 codegen allows at most ONE sync-wait command per engine
    instruction (S3D3_*_STRUCT).  Tile's wait assignment can emit 2+ — move
    the excess onto standalone InstEventSemaphore no-ops (same engine, placed
    immediately before), which is the same mechanism Tile's own barriers use.
    Each carries a dead increment of ``dummy_sem`` (CoreSim requires updates).
    """
    from concourse import mybir

    n = 0
    for fn in nc.m.functions:
        for blk in fn.blocks:
            out = []
            changed = False
            for inst in blk.instructions:
                si = inst.sync_info
                if si is not None and si.on_wait and len(si.on_wait) > 1:
                    waits = list(si.on_wait)
                    for w in waits[:-1]:
                        n += 1
                        ev = mybir.InstEventSemaphore(
                            name=f"I-evsplit-{n}", ins=[], outs=[])
                        ev.engine = inst.engine
                        ev.sync_info = mybir.SyncInfo(
                            on_wait=[w],
                            on_update=[mybir.SyncUpdate(
                                sync_type="semaphore", id=dummy_sem.num,
                                ant_name=dummy_sem.name,
                                update_mode="sem-inc", update_value=1,
                                update_reg=None)])
                        out.append(ev)
                    inst.sync_info = mybir.SyncInfo(
                        on_wait=[waits[-1]], on_update=list(si.on_update or []))
                    changed = True
                out.append(inst)
            if changed:
                blk.instructions = out
    return n


def build_nc(rows_pad, g, data_bufs=8, ss_split=SS_SPLIT):
    import concourse.bacc as bacc
    import concourse.tile as tile
    from concourse import mybir
    # Loads issue from the SP HWDGE ring; stores from the ACT HWDGE ring so
    # the two directions run on independent FIFO rings and GPSIMD (whose
    # SWDGE descriptor generation would contend with DVE on the shared SBUF
    # port) stays idle. Pin one completion-semaphore lane per ring.
    from concourse import tile_sem_assignment as _tsa
    if not getattr(_tsa.TileClockTick, "_lane_by_engine", False):
        _orig_assign = _tsa.TileClockTick._assign_tick

        def _assign_tick_lane_by_engine(self, inst):
            if isinstance(inst, _tsa.DMAInst) and not isinstance(
                    inst, _tsa.bass_isa.UserSyncedRemoteDMADescs):
                if inst.engine == mybir.EngineType.SP:
                    self.next_hw_dma_idx = 0
                elif inst.engine == mybir.EngineType.Activation:
                    self.next_hw_dma_idx = 1
            return _orig_assign(self, inst)

        _tsa.TileClockTick._assign_tick = _assign_tick_lane_by_engine
        _tsa.TileClockTick._lane_by_engine = True

    f16 = mybir.dt.float16
    f32 = mybir.dt.float32
    Alu = mybir.AluOpType
    Act = mybir.ActivationFunctionType

    nblocks = rows_pad // BLK
    assert rows_pad % BLK == 0
    if isinstance(g, int):
        assert nblocks % g == 0
        gs = [g] * (nblocks // g)
    else:
        gs = list(g)
        assert sum(gs) == nblocks
    ntiles = len(gs)
    r0s = list(np.cumsum([0] + gs[:-1]) * BLK)

    nc = bacc.Bacc("TRN2", target_bir_lowering=False, debug=False)
    evsem = nc.alloc_semaphore("evsplit_dummy")
    x = nc.dram_tensor("x", [rows_pad, DIM], f16, kind="ExternalInput").ap()
    # w/b arrive pre-replicated across the 128 partitions (host-side tile):
    # a plain contiguous load is ~20x faster than a broadcast DMA (128 tiny
    # descriptors) and keeps the ACT ring free for the ramp loads behind it.
    w = nc.dram_tensor("w", [BLK, DIM], f16, kind="ExternalInput").ap()
    b = nc.dram_tensor("b", [BLK, NS], f16, kind="ExternalInput").ap()
    y = nc.dram_tensor("y", [rows_pad, DIM], f16, kind="ExternalOutput").ap()

    with tile.TileContext(nc) as tc:
        with (
            tc.tile_pool(name="const", bufs=1) as cpool,
            tc.tile_pool(name="data", bufs=data_bufs) as dpool,
            tc.tile_pool(name="stats", bufs=ntiles) as spool,
            tc.tile_pool(name="scratch", bufs=1) as zpool,
            tc.tile_pool(name="sq", bufs=3) as qpool,
        ):
            # One-time const loads on the (initially idle) ACT ring so they
            # don't clog the SP ring ahead of the first data tiles.
            w_t = cpool.tile([BLK, DIM], f16, name="w_t")
            nc.scalar.dma_start(out=w_t[:], in_=w)
            b_t = cpool.tile([BLK, NS], f16, name="b_t")
            nc.scalar.dma_start(out=b_t[:], in_=b)

            df = zpool.tile([BLK, DIM], f16, name="df")   # ACT dead (squares)
            da = zpool.tile([BLK, DIM], f16, name="da")   # DVE dead (TS accum)

            T = [dict() for _ in range(ntiles)]

            def emit_load(t):
                g = gs[t]
                src = x[r0s[t]:r0s[t] + g * BLK, :].rearrange(
                    "(p g) d -> p g d", p=BLK)
                xt = dpool.tile([BLK, g, DIM], f16, tag="xt", name=f"xt{t}")
                nc.sync.dma_start(out=xt[:], in_=src)
                T[t]["xt"] = xt

            def emit_reduce_nm(t):
                g, xt = gs[t], T[t]["xt"]
                s1r = spool.tile([BLK, g], f32, tag="s1r", name=f"s1r{t}")
                nc.vector.tensor_reduce(
                    out=s1r[:], in_=xt[:, :, 0:NS],
                    axis=mybir.AxisListType.X, op=Alu.add)
                nm = spool.tile([BLK, g], f32, tag="nm", name=f"nm{t}")
                nc.vector.tensor_scalar(
                    out=nm[:], in0=s1r[:], scalar1=-1.0 / NS, scalar2=None,
                    op0=Alu.mult)
                T[t]["nm"] = nm

            def emit_accums(t):
                g, xt = gs[t], T[t]["xt"]
                ss = spool.tile([BLK, g], f32, tag="ss", name=f"ss{t}")
                k = min(ss_split, g)
                for j in range(g - k):
                    nc.scalar.activation(
                        out=df[:], in_=xt[:, j, :], func=Act.Square,
                        scale=1.0 / float(NF) ** 0.5,
                        accum_out=ss[:, j:j + 1])
                if k > 0:
                    xsq = qpool.tile([BLK, k, DIM], f16, tag="xsq",
                                     name=f"xsq{t}")
                    nc.scalar.activation(
                        out=xsq[:], in_=xt[:, g - k:, :], func=Act.Square,
                        scale=1.0 / float(NF) ** 0.5)
                    T[t]["xsq"] = xsq
                T[t]["ss"] = ss
                T[t]["k"] = k

            def emit_mid(t):
                g, nm, ss, k = gs[t], T[t]["nm"], T[t]["ss"], T[t]["k"]
                for j in range(k):
                    # op1 is the reduce op when accum_out is given
                    nc.vector.tensor_scalar(
                        out=da[:], in0=T[t]["xsq"][:, j, :], scalar1=1.0,
                        scalar2=None, op0=Alu.mult, op1=Alu.add,
                        accum_out=ss[:, g - k + j:g - k + j + 1])
                # vv = ss/224 - (128/224)*nm^2 ; rr = 1/sqrt(vv)
                u = spool.tile([BLK, g], f32, tag="u", name=f"u{t}")
                nc.vector.scalar_tensor_tensor(
                    out=u[:], in0=nm[:], scalar=-float(NS) / float(NF),
                    in1=nm[:], op0=Alu.mult, op1=Alu.mult)
                vv = spool.tile([BLK, g], f32, tag="vv", name=f"vv{t}")
                nc.vector.tensor_tensor(
                    out=vv[:], in0=ss[:], in1=u[:], op=Alu.add)
                sq = spool.tile([BLK, g], f32, tag="sq", name=f"sq{t}")
                nc.scalar.activation(out=sq[:], in_=vv[:], func=Act.Sqrt)
                rr = spool.tile([BLK, g], f32, tag="rr", name=f"rr{t}")
                nc.vector.reciprocal(out=rr[:], in_=sq[:])
                T[t]["rr"] = rr

            def emit_scale(t):
                g, xt, nm, rr = gs[t], T[t]["xt"], T[t]["nm"], T[t]["rr"]
                # Chunk A: (x + nm) * rr fused in one 2-scalar TS;
                # chunks B/C: x * rr. Both 2x fp16 on DVE.
                for j in range(g):
                    nc.vector.tensor_scalar(
                        out=xt[:, j, 0:NS], in0=xt[:, j, 0:NS],
                        scalar1=nm[:, j:j + 1], scalar2=rr[:, j:j + 1],
                        op0=Alu.add, op1=Alu.mult)
                    nc.vector.tensor_scalar(
                        out=xt[:, j, NS:DIM], in0=xt[:, j, NS:DIM],
                        scalar1=rr[:, j:j + 1], scalar2=None, op0=Alu.mult)
                # Whole-tile *w, then whole-tile +bias on the scalar chunk.
                w_bc = w_t[:].unsqueeze(1).broadcast_to([BLK, g, DIM])
                nc.vector.tensor_tensor(
                    out=xt[:], in0=xt[:], in1=w_bc, op=Alu.mult)
                b_bc = b_t[:].unsqueeze(1).broadcast_to([BLK, g, NS])
                nc.vector.tensor_tensor(
                    out=xt[:, :, 0:NS], in0=xt[:, :, 0:NS], in1=b_bc,
                    op=Alu.add)

            def emit_store(t):
                # SWDGE store: GPSIMD has no other work, so the store's wait
                # for the tile's last DVE op costs nothing — a waiting store
                # on the FIFO ACT ring would stall the accums/sqrt behind it.
                g, xt = gs[t], T[t]["xt"]
                dst = y[r0s[t]:r0s[t] + g * BLK, :].rearrange(
                    "(p g) d -> p g d", p=BLK)
                nc.gpsimd.dma_start(out=dst, in_=xt[:])

            # Software-pipelined emission: per-engine instruction order
            # follows emission order, so putting tile t+1's load/reduce
            # ahead of tile t's mid/scale keeps both engines fed.
            emit_load(0)
            emit_reduce_nm(0)
            for t in range(ntiles):
                if t + 1 < ntiles:
                    emit_load(t + 1)
                    emit_reduce_nm(t + 1)
                emit_accums(t)
                emit_mid(t)
                emit_scale(t)
                emit_store(t)

    nc.compile()
    _split_excess_waits(nc, evsem)
    return nc


_NC_CACHE = {}


def _get_nc(rows_pad, g):
    key = (rows_pad, tuple(g) if not isinstance(g, int) else g)
    if key not in _NC_CACHE:
        _NC_CACHE[key] = build_nc(rows_pad, g)
    return _NC_CACHE[key]


def kernel(node_input, affine_weight, affine_bias):
    from concourse.bass_utils import run_bass_kernel_spmd

    x16 = np.asarray(node_input).astype(np.float16)
    assert x16.shape == (N_TOTAL, DIM)
    w16 = np.tile(_expand_w(np.asarray(affine_weight, dtype=np.float32)).astype(
        np.float16).reshape(1, DIM), (BLK, 1))
    b16 = np.tile(np.asarray(affine_bias).astype(np.float16).reshape(1, NS),
                  (BLK, 1))

    in_maps = []
    for c in range(N_CORES):
        shard = np.ones((ROWS_PAD, DIM), dtype=np.float16)
        shard[:ROWS_PER_CORE] = x16[c * ROWS_PER_CORE:(c + 1) * ROWS_PER_CORE]
        in_maps.append({"x": shard, "w": w16, "b": b16})

    nc = _get_nc(ROWS_PAD, G_SCHEDULE)
    res = run_bass_kernel_spmd(nc, in_maps, core_ids=list(range(N_CORES)))
    out = np.concatenate(
        [np.asarray(res.results[c]["y"])[:ROWS_PER_CORE] for c in range(N_CORES)],
        axis=0)
    return out.astype(np.float32)


# revision 24
# speedup vs baseline: 1.0992x; 1.0343x over previous
"""Trainium2 Bass kernel: EquivariantLayerNorm (irreps 128x0e + 64x1o + 32x2e).

Math (per row x of 480 features; scalar channels = first 128):
    m    = mean(x[:128]);  x'[:128] = x[:128] - m;  x'[128:] = x[128:]
    ss   = sum(x'^2) = sum(x^2) - 128*m^2           (groups partition the row)
    r    = 1/sqrt(ss/224)
    y    = x' * r * w_full;  y[:128] += bias
The Invariant eps terms (eps=1e-6) contribute <1e-6 relative error and are
dropped (below fp32 rounding of the reference itself).

I/O in fp16: the host casts inputs to fp16 and upconverts the fp16 output;
this halves HBM traffic (the kernel is memory-bound) and lands well inside
the 2e-2 relative-error budget (~9e-4 measured on HW).

Sharding: pure data-parallel over rows across 8 NeuronCores; each core gets
12500 rows padded to 12544 = 98 blocks of 128 rows (pad rows 1.0).

Per-core layout: rows on partitions, features on the free dim. Tiles of g
row-blocks per DMA (G_SCHEDULE), mapped "(p g) d -> p g d" so each partition
holds g CONSECUTIVE HBM rows: one contiguous ~g*960B descriptor per
partition per tile instead of g small ones (8x fewer DMA descriptors; loads
were descriptor-bound at ~98 GB/s with the interleaved mapping).

GPSIMD is kept COMPLETELY IDLE: profiling showed GPSIMD activity (elementwise
TTs + SWDGE descriptor generation) contends with DVE on the shared SBUF port
and inflates concurrent DVE ops up to 8x. Loads issue on the SP HWDGE ring,
stores (and the one-time w/b broadcasts) on the ACT HWDGE ring.

Emission is software-pipelined (stats front-half of tile t+1 before the
scale phase of tile t) so neither DVE nor ACT stalls on the cross-engine
stats chain (ACT square-accums -> DVE vv -> ACT sqrt -> DVE recip):
  DVE : s1 rowsum (one 3D tensor_reduce per tile); per-block 2-scalar
        tensor_scalar (x+nm)*rr on the scalar chunk and *rr on the rest
        (2x fp16); whole-tile tensor_tensor *w and +bias (2x fp16);
        small stats ops; reciprocal; ss for SS_SPLIT blocks/tile.
  ACT : per-block Square-accumulate -> sum(x^2)/224 for the other blocks;
        batched Square for the DVE-split blocks; sqrt; store dma_start.
  SP  : input loads (HWDGE).
"""

import numpy as np

DIM = 480
NS = 128          # scalar (0e) channels
NF = 224          # irrep instances
BLK = 128         # rows per block (partition dim)
N_CORES = 8

N_TOTAL = 100000
ROWS_PER_CORE = N_TOTAL // N_CORES    # 12500
NBLOCKS = 98                          # ceil(12500/128)
ROWS_PAD = NBLOCKS * BLK              # 12544

# Variable tile schedule: small first tiles start compute early; small last
# tiles trim the drain tail (the final tile's serial chain).
G_SCHEDULE = (6,) + (8,) * 11 + (4,)   # sums to 98
SS_SPLIT = 1     # blocks per tile whose sum-sq runs on DVE (ACT Square into
                 # xsq scratch, then DVE tensor_scalar-accumulate); the rest
                 # use ACT Square-accumulate. Balances ACT/DVE load.


def _expand_w(affine_weight):
    return np.concatenate([
        affine_weight[0:128],
        np.repeat(affine_weight[128:192], 3),
        np.repeat(affine_weight[192:224], 5),
    ]).astype(np.float32)


def _split_excess_waits(nc, dummy_sem):
    """walrus' TRN2 codegen allows at most ONE sync-wait command per engine
    instruction (S3D3_*_STRUCT).  Tile's wait assignment can emit 2+ — move
    the excess onto standalone InstEventSemaphore no-ops (same engine, placed
    immediately before), which is the same mechanism Tile's own barriers use.
    Each carries a dead increment of ``dummy_sem`` (CoreSim requires updates).
    """
    from concourse import mybir

    n = 0
    for fn in nc.m.functions:
        for blk in fn.blocks:
            out = []
            changed = False
            for inst in blk.instructions:
                si = inst.sync_info
                if si is not None and si.on_wait and len(si.on_wait) > 1:
                    waits = list(si.on_wait)
                    for w in waits[:-1]:
                        n += 1
                        ev = mybir.InstEventSemaphore(
                            name=f"I-evsplit-{n}", ins=[], outs=[])
                        ev.engine = inst.engine
                        ev.sync_info = mybir.SyncInfo(
                            on_wait=[w],
                            on_update=[mybir.SyncUpdate(
                                sync_type="semaphore", id=dummy_sem.num,
                                ant_name=dummy_sem.name,
                                update_mode="sem-inc", update_value=1,
                                update_reg=None)])
                        out.append(ev)
                    inst.sync_info = mybir.SyncInfo(
                        on_wait=[waits[-1]], on_update=list(si.on_update or []))
                    changed = True
                out.append(inst)
            if changed:
                blk.instructions = out
    return n


def build_nc(rows_pad, g, data_bufs=8, ss_split=SS_SPLIT):
    import concourse.bacc as bacc
    import concourse.tile as tile
    from concourse import mybir
    # Loads issue from the SP HWDGE ring; stores from the ACT HWDGE ring so
    # the two directions run on independent FIFO rings and GPSIMD (whose
    # SWDGE descriptor generation would contend with DVE on the shared SBUF
    # port) stays idle. Pin one completion-semaphore lane per ring.
    from concourse import tile_sem_assignment as _tsa
    if not getattr(_tsa.TileClockTick, "_lane_by_engine", False):
        _orig_assign = _tsa.TileClockTick._assign_tick

        def _assign_tick_lane_by_engine(self, inst):
            if isinstance(inst, _tsa.DMAInst) and not isinstance(
                    inst, _tsa.bass_isa.UserSyncedRemoteDMADescs):
                if inst.engine == mybir.EngineType.SP:
                    self.next_hw_dma_idx = 0
                elif inst.engine == mybir.EngineType.Activation:
                    self.next_hw_dma_idx = 1
            return _orig_assign(self, inst)

        _tsa.TileClockTick._assign_tick = _assign_tick_lane_by_engine
        _tsa.TileClockTick._lane_by_engine = True

    f16 = mybir.dt.float16
    f32 = mybir.dt.float32
    Alu = mybir.AluOpType
    Act = mybir.ActivationFunctionType

    nblocks = rows_pad // BLK
    assert rows_pad % BLK == 0
    if isinstance(g, int):
        assert nblocks % g == 0
        gs = [g] * (nblocks // g)
    else:
        gs = list(g)
        assert sum(gs) == nblocks
    ntiles = len(gs)
    r0s = list(np.cumsum([0] + gs[:-1]) * BLK)

    nc = bacc.Bacc("TRN2", target_bir_lowering=False, debug=False)
    evsem = nc.alloc_semaphore("evsplit_dummy")
    x = nc.dram_tensor("x", [rows_pad, DIM], f16, kind="ExternalInput").ap()
    # w/b arrive pre-replicated across the 128 partitions (host-side tile):
    # a plain contiguous load is ~20x faster than a broadcast DMA (128 tiny
    # descriptors) and keeps the ACT ring free for the ramp loads behind it.
    w = nc.dram_tensor("w", [BLK, DIM], f16, kind="ExternalInput").ap()
    b = nc.dram_tensor("b", [BLK, NS], f16, kind="ExternalInput").ap()
    y = nc.dram_tensor("y", [rows_pad, DIM], f16, kind="ExternalOutput").ap()

    with tile.TileContext(nc) as tc:
        with (
            tc.tile_pool(name="const", bufs=1) as cpool,
            tc.tile_pool(name="data", bufs=data_bufs) as dpool,
            tc.tile_pool(name="stats", bufs=ntiles) as spool,
            tc.tile_pool(name="scratch", bufs=1) as zpool,
            tc.tile_pool(name="sq", bufs=3) as qpool,
        ):
            # One-time const loads on the (initially idle) ACT ring so they
            # don't clog the SP ring ahead of the first data tiles.
            w_t = cpool.tile([BLK, DIM], f16, name="w_t")
            nc.scalar.dma_start(out=w_t[:], in_=w)
            b_t = cpool.tile([BLK, NS], f16, name="b_t")
            nc.scalar.dma_start(out=b_t[:], in_=b)

            df = zpool.tile([BLK, DIM], f16, name="df")   # ACT dead (squares)
            da = zpool.tile([BLK, DIM], f16, name="da")   # DVE dead (TS accum)

            T = [dict() for _ in range(ntiles)]

            def emit_load(t):
                g = gs[t]
                src = x[r0s[t]:r0s[t] + g * BLK, :].rearrange(
                    "(p g) d -> p g d", p=BLK)
                xt = dpool.tile([BLK, g, DIM], f16, tag="xt", name=f"xt{t}")
                nc.sync.dma_start(out=xt[:], in_=src)
                T[t]["xt"] = xt

            def emit_reduce_nm(t):
                g, xt = gs[t], T[t]["xt"]
                s1r = spool.tile([BLK, g], f32, tag="s1r", name=f"s1r{t}")
                nc.vector.tensor_reduce(
                    out=s1r[:], in_=xt[:, :, 0:NS],
                    axis=mybir.AxisListType.X, op=Alu.add)
                nm = spool.tile([BLK, g], f32, tag="nm", name=f"nm{t}")
                nc.vector.tensor_scalar(
                    out=nm[:], in0=s1r[:], scalar1=-1.0 / NS, scalar2=None,
                    op0=Alu.mult)
                T[t]["nm"] = nm
                # GPSIMD centers the scalar chunk in place BEFORE the ACT
                # square-accums read it: the accumulated ss then directly
                # equals the variance term (sum(x-m)^2_A + sum(x^2)_BC)/224,
                # killing the -128m^2 correction ops, and DVE's per-block
                # scale collapses to a single full-row *rr. One 2-input TT
                # per tile (~30% GPS busy) — runs a tile ahead, off the
                # critical path.
                nm_bc = nm[:].unsqueeze(2).broadcast_to([BLK, g, NS])
                nc.gpsimd.tensor_tensor(
                    out=xt[:, :, 0:NS], in0=xt[:, :, 0:NS], in1=nm_bc,
                    op=Alu.add)

            def emit_accums(t):
                g, xt = gs[t], T[t]["xt"]
                ss = spool.tile([BLK, g], f32, tag="ss", name=f"ss{t}")
                k = min(ss_split, g)
                for j in range(g - k):
                    nc.scalar.activation(
                        out=df[:], in_=xt[:, j, :], func=Act.Square,
                        scale=1.0 / float(NF) ** 0.5,
                        accum_out=ss[:, j:j + 1])
                if k > 0:
                    xsq = qpool.tile([BLK, k, DIM], f16, tag="xsq",
                                     name=f"xsq{t}")
                    nc.scalar.activation(
                        out=xsq[:], in_=xt[:, g - k:, :], func=Act.Square,
                        scale=1.0 / float(NF) ** 0.5)
                    T[t]["xsq"] = xsq
                T[t]["ss"] = ss
                T[t]["k"] = k

            def emit_mid(t):
                g, ss, k = gs[t], T[t]["ss"], T[t]["k"]
                for j in range(k):
                    # op1 is the reduce op when accum_out is given
                    nc.vector.tensor_scalar(
                        out=da[:], in0=T[t]["xsq"][:, j, :], scalar1=1.0,
                        scalar2=None, op0=Alu.mult, op1=Alu.add,
                        accum_out=ss[:, g - k + j:g - k + j + 1])
                # Chunk A was centered before the squares, so ss IS the
                # variance term already: rr = 1/sqrt(ss).
                sq = spool.tile([BLK, g], f32, tag="sq", name=f"sq{t}")
                nc.scalar.activation(out=sq[:], in_=ss[:], func=Act.Sqrt)
                rr = spool.tile([BLK, g], f32, tag="rr", name=f"rr{t}")
                nc.vector.reciprocal(out=rr[:], in_=sq[:])
                T[t]["rr"] = rr

            def emit_scale(t):
                g, xt, rr = gs[t], T[t]["xt"], T[t]["rr"]
                # One full-row *rr per block (2x fp16); chunk A is already
                # mean-centered in place by GPSIMD.
                for j in range(g):
                    nc.vector.tensor_scalar(
                        out=xt[:, j, :], in0=xt[:, j, :],
                        scalar1=rr[:, j:j + 1], scalar2=None, op0=Alu.mult)
                # Whole-tile *w, then whole-tile +bias on the scalar chunk.
                w_bc = w_t[:].unsqueeze(1).broadcast_to([BLK, g, DIM])
                nc.vector.tensor_tensor(
                    out=xt[:], in0=xt[:], in1=w_bc, op=Alu.mult)
                b_bc = b_t[:].unsqueeze(1).broadcast_to([BLK, g, NS])
                nc.vector.tensor_tensor(
                    out=xt[:, :, 0:NS], in0=xt[:, :, 0:NS], in1=b_bc,
                    op=Alu.add)

            def emit_store(t):
                # SWDGE store: GPSIMD has no other work, so the store's wait
                # for the tile's last DVE op costs nothing — a waiting store
                # on the FIFO ACT ring would stall the accums/sqrt behind it.
                g, xt = gs[t], T[t]["xt"]
                dst = y[r0s[t]:r0s[t] + g * BLK, :].rearrange(
                    "(p g) d -> p g d", p=BLK)
                nc.gpsimd.dma_start(out=dst, in_=xt[:])

            # Software-pipelined emission: per-engine instruction order
            # follows emission order, so putting tile t+1's load/reduce
            # ahead of tile t's mid/scale keeps both engines fed.
            emit_load(0)
            emit_reduce_nm(0)
            for t in range(ntiles):
                if t + 1 < ntiles:
                    emit_load(t + 1)
                    emit_reduce_nm(t + 1)
                emit_accums(t)
                emit_mid(t)
                emit_scale(t)
                emit_store(t)

    nc.compile()
    _split_excess_waits(nc, evsem)
    return nc


_NC_CACHE = {}


def _get_nc(rows_pad, g):
    key = (rows_pad, tuple(g) if not isinstance(g, int) else g)
    if key not in _NC_CACHE:
        _NC_CACHE[key] = build_nc(rows_pad, g)
    return _NC_CACHE[key]


def kernel(node_input, affine_weight, affine_bias):
    from concourse.bass_utils import run_bass_kernel_spmd

    x16 = np.asarray(node_input).astype(np.float16)
    assert x16.shape == (N_TOTAL, DIM)
    w16 = np.tile(_expand_w(np.asarray(affine_weight, dtype=np.float32)).astype(
        np.float16).reshape(1, DIM), (BLK, 1))
    b16 = np.tile(np.asarray(affine_bias).astype(np.float16).reshape(1, NS),
                  (BLK, 1))

    in_maps = []
    for c in range(N_CORES):
        shard = np.ones((ROWS_PAD, DIM), dtype=np.float16)
        shard[:ROWS_PER_CORE] = x16[c * ROWS_PER_CORE:(c + 1) * ROWS_PER_CORE]
        in_maps.append({"x": shard, "w": w16, "b": b16})

    nc = _get_nc(ROWS_PAD, G_SCHEDULE)
    res = run_bass_kernel_spmd(nc, in_maps, core_ids=list(range(N_CORES)))
    out = np.concatenate(
        [np.asarray(res.results[c]["y"])[:ROWS_PER_CORE] for c in range(N_CORES)],
        axis=0)
    return out.astype(np.float32)
